# revision 9
# baseline (speedup 1.0000x reference)
"""Mat2Twist Trainium2 kernel: batch of 3x3 rotation matrices -> twist vectors.

For each matrix R:  tr = trace(R); x = (tr-1)/2 = cos(theta)
  theta = arccos(x);  w = [R21-R12, R02-R20, R10-R01]  (|w| = 2 sin theta)
  out = theta/(2 sin theta) * w

arccos via the Abramowitz-Stegun-style split
  arccos(x) = pi/2 - sign(x)*arcsin(|x|),
  arcsin(|x|) = pi/2 - sqrt(1-|x|) * q(|x|),  q deg-2 minimax (err ~1e-3)
so every activation (Abs, Sign, Square, Sqrt) lives in ONE ACT table set
("sqrt_and_others") -- the old Square/Ln/Exp/Arctan mix forced ~2
ACT_TABLE_LOADs (1.3us each) per chunk and made the scalar engine the
pipeline straggler.  1/sin(theta) is a magic-constant bitwise seed plus
one tuned Newton step on plain DVE ALU ops (rel err ~1.5e-3; output
gate is 2e-2).  All sign flips fold into existing scale/scalar slots.

Per chunk (m matrices per partition), tile X = [minu(3m)|subt(3m)|R00|R11|R22]:
  gp:  t = R00+R11; tr' = (t-1)+R22 = tr-1 = 2x     [GpSimd TT+STT]
  act: ax=Abs(.5tr') sg=Sign(-.5tr')=-sign(x) v=Square(.5tr')
       qq=Square(SQ*ax+B2)  sn=Sqrt(1-v)  S=Sqrt(1-ax)  [6 ACT, one set]
  dve: w = minu-subt (in place)
       y0 = bitcast(MAGIC - bits(sn))  = ~1/sn seed    [int ts]
       t  = sn*y0;  mrs = (t-K2)*y0   = -1/sn          [TT+STT]
       u  = (qq+KQ)*S       = arccos(|x|)
       p  = (u-pi/2)*sg     = pi/2 - theta
       P  = (p-pi/2)*mrs    = theta/sin(theta)
       out_k = (0.5*P)*w_k  as one STT over [P,3,m], P broadcast
  out-DMA on the ACT ring, inputs on the SP ring.

Software-pipelined emission, skew: dma(i)@i -> trace@i+1 -> acts@i+2 ->
dve@i+3 -> out-dma trigger@i+4 (trigger emitted at the top of its
iteration so it never queues behind that iteration's ACT block).  Tail
chunks shrink (256,128,128) to cut pipeline-drain latency after the
last input byte lands.
"""

import numpy as np

import concourse.bass as bass
import concourse.mybir as mybir
from concourse.tile import TileContext
from concourse.bass_utils import run_bass_kernel_spmd

B = 4194304
NCORES = 8
P = 128
N_C = B // NCORES        # 524288 matrices per core
MPP = N_C // P           # 4096 matrices per partition
MS = [512] * 7 + [256] + [128] * 2   # per-chunk matrices per partition
assert sum(MS) == MPP

# component order in DRAM (flat 3x3 index): minuends, subtrahends, diagonal
PERM = [7, 2, 3, 5, 6, 1, 0, 4, 8]

F32 = mybir.dt.float32
I32 = mybir.dt.int32
ACT = mybir.ActivationFunctionType
ALU = mybir.AluOpType
PI_2 = float(np.pi / 2.0)
MAXM = max(MS)

# deg-2 minimax fit of q(t) = arccos(t)/sqrt(1-t) on [0, cos(0.1)],
# written as q = (SQ*t + B2)^2 + KQ so one ACT Square evaluates it.
SQ = 0.21443806949144176
B2 = -0.46791288034992956
KQ = 1.3508104959051634
# reciprocal magic seed (y0_bits = MAGIC - x_bits) + tuned Newton const,
# optimized for x in [sin 0.1, 1]: rel err 1.5e-3.
MAGIC_P1 = 0x7EF28000 + 1   # passed as (x ^ -1) + (MAGIC+1) == MAGIC - x
K2 = 2.0015


def _split_multi_waits(nc):
    """This container's walrus build rejects >1 sem-wait per instruction
    ("Too many sync wait commands"); hoist extras onto preceding NOPs."""
    for f in nc.m.functions:
        for blk in f.blocks:
            il = blk.instructions
            new = []
            for ins in il:
                si = ins.sync_info
                if si is not None and si.on_wait is not None and len(si.on_wait) > 1:
                    waits = list(si.on_wait)
                    for j, w in enumerate(waits[:-1]):
                        nop = mybir.InstNoOp(name=f"{ins.name}-ws{j}", engine=ins.engine)
                        nop.sync_info = mybir.SyncInfo(on_wait=[w], on_update=[])
                        new.append(nop)
                    ins.sync_info = mybir.SyncInfo(
                        on_wait=[waits[-1]], on_update=list(si.on_update or [])
                    )
                new.append(ins)
            il[:] = new


def _build_kernel():
    nc = bass.Bass()
    # extra const APs (memsets + one barrier before TileContext, same as
    # the built-ins): activation biases, int32 reciprocal magic.
    for val in (B2, -0.5, 0.5):
        t = nc.alloc_sbuf_tensor(f"const-f32-{val}", [128, 1], F32)
        nc.gpsimd.memset(t.ap(), val)
        nc.const_aps.aps[(F32, val)] = t.ap()
    magic = nc.alloc_sbuf_tensor("recip-magic", [128, 1], I32)
    nc.gpsimd.memset(magic.ap(), MAGIC_P1)
    nc.all_engine_barrier()
    magic_ap = magic.ap()

    x_in = nc.dram_tensor("mat_in", [N_C * 9], F32, kind="ExternalInput")
    y_out = nc.dram_tensor("twist_out", [N_C * 3], F32, kind="ExternalOutput")

    n = len(MS)
    offs = [0] + list(np.cumsum(MS)[:-1].astype(int))

    with TileContext(nc) as tc:
        with tc.tile_pool(name="xp", bufs=8) as xp, \
             tc.tile_pool(name="tsm", bufs=3) as tsm, \
             tc.tile_pool(name="trs", bufs=2) as trs:

            X_, tr_, ax_, sg_, v_, qq_ = {}, {}, {}, {}, {}, {}

            def dma_in(i):
                m = MS[i]
                base = offs[i] * P * 9
                X_[i] = xp.tile([P, 9 * MAXM], F32, tag="X", name=f"X{i}")[:, : 9 * m]
                nc.sync.dma_start(
                    out=X_[i],
                    in_=x_in[base : base + P * 9 * m].rearrange("(p n) -> p n", p=P),
                )

            def gp_trace(i):
                m = MS[i]
                X = X_[i]
                tr = tsm.tile([P, MAXM], F32, tag="tr", name=f"tr{i}")[:, :m]
                nc.gpsimd.tensor_add(
                    out=tr, in0=X[:, 6 * m : 7 * m], in1=X[:, 7 * m : 8 * m]
                )
                nc.gpsimd.tensor_add(out=tr, in0=tr, in1=X[:, 8 * m : 9 * m])
                tr_[i] = tr

            def act_block(i):
                m = MS[i]
                tr = tr_[i]
                ax = tsm.tile([P, MAXM], F32, tag="ax", name=f"ax{i}")[:, :m]
                sg = tsm.tile([P, MAXM], F32, tag="sg", name=f"sg{i}")[:, :m]
                v = tsm.tile([P, MAXM], F32, tag="v", name=f"v{i}")[:, :m]
                qq = tsm.tile([P, MAXM], F32, tag="qq", name=f"qq{i}")[:, :m]
                nc.scalar.activation(ax, tr, ACT.Abs, scale=0.5, bias=-0.5)
                nc.scalar.activation(sg, tr, ACT.Sign, scale=-0.5, bias=0.5)
                nc.scalar.activation(v, tr, ACT.Square, scale=0.5, bias=-0.5)
                nc.scalar.activation(qq, ax, ACT.Square, scale=SQ, bias=B2)
                # in place: sn over v, S over ax (qq already consumed ax)
                nc.scalar.activation(v, v, ACT.Sqrt, scale=-1.0, bias=1.0)
                nc.scalar.activation(ax, ax, ACT.Sqrt, scale=-1.0, bias=1.0)
                ax_[i], sg_[i], v_[i], qq_[i] = ax, sg, v, qq

            def dve_block(i):
                m = MS[i]
                X = X_[i]
                # w = minu - subt, in place in X
                nc.vector.tensor_sub(
                    out=X[:, 0 : 3 * m], in0=X[:, 0 : 3 * m], in1=X[:, 3 * m : 6 * m]
                )
                sn = v_[i]
                y0 = trs.tile([P, MAXM], F32, tag="y0", name=f"y0{i}")[:, :m]
                # y0 = bitcast((bits(sn) ^ -1) + (MAGIC+1)) = MAGIC - bits(sn)
                # ~= 1/sn seed.  walrus forbids bitwise+arith in one op and
                # f32-types arithmetic scalar slots (lossy for big ints), so:
                # xor as a scalar op, then an int add against a stride-0
                # int32 broadcast of the magic constant.
                y0i = y0.bitcast(I32)
                nc.vector.tensor_scalar(
                    out=y0i, in0=sn.bitcast(I32), scalar1=-1, scalar2=None,
                    op0=ALU.bitwise_xor,
                )
                nc.vector.tensor_add(
                    out=y0i, in0=y0i, in1=magic_ap.to_broadcast((P, m))
                )
                # t = sn*y0 (over sn); mrs = (t - K2)*y0 = -1/sn (over y0)
                nc.vector.tensor_mul(out=sn, in0=sn, in1=y0)
                nc.vector.scalar_tensor_tensor(
                    out=y0, in0=sn, scalar=K2, in1=y0,
                    op0=ALU.subtract, op1=ALU.mult,
                )
                qq = qq_[i]
                # u = (qq + KQ) * S = arccos(|x|)
                nc.vector.scalar_tensor_tensor(
                    out=qq, in0=qq, scalar=KQ, in1=ax_[i],
                    op0=ALU.add, op1=ALU.mult,
                )
                # p = (u - pi/2) * (-sign x) = pi/2 - theta
                nc.vector.scalar_tensor_tensor(
                    out=qq, in0=qq, scalar=PI_2, in1=sg_[i],
                    op0=ALU.subtract, op1=ALU.mult,
                )
                # P = (p - pi/2) * (-1/sn) = theta / sin(theta)
                nc.vector.scalar_tensor_tensor(
                    out=qq, in0=qq, scalar=PI_2, in1=y0,
                    op0=ALU.subtract, op1=ALU.mult,
                )
                # out_k = (0.5 * P) * w_k, P broadcast over the 3 w-blocks
                bcast = qq.rearrange("p (o n) -> p o n", o=1).to_broadcast((P, 3, m))
                w3 = X[:, 0 : 3 * m].rearrange("p (k n) -> p k n", k=3)
                nc.vector.scalar_tensor_tensor(
                    out=w3, in0=bcast, scalar=0.5, in1=w3,
                    op0=ALU.mult, op1=ALU.mult,
                )

            def out_dma(i):
                m = MS[i]
                dst = y_out[offs[i] * P * 3 : (offs[i] + m) * P * 3].rearrange(
                    "(p n) -> p n", p=P
                )
                nc.scalar.dma_start(out=dst, in_=X_[i][:, 0 : 3 * m])

            def valid(j):
                return 0 <= j < n

            # software-pipelined emission; skew in iterations:
            # dma(i)@i, trace@i+1, acts@i+2, dve@i+3, out-dma@i+4 (trigger
            # emitted at the top of the iteration so the ACT-ring trigger
            # never queues behind that iteration's ACT block)
            for i in range(n + 4):
                if valid(i):
                    dma_in(i)
                if valid(i - 4):
                    out_dma(i - 4)
                if valid(i - 1):
                    gp_trace(i - 1)
                if valid(i - 3):
                    dve_block(i - 3)
                if valid(i - 2):
                    act_block(i - 2)

    _split_multi_waits(nc)
    return nc


_NC_CACHE = []


def _host_pack(mat_batch: np.ndarray) -> np.ndarray:
    """[B,3,3] -> [NCORES, N_C*9] tile-major/component-major PERM layout."""
    flat = np.ascontiguousarray(mat_batch, dtype=np.float32).reshape(
        NCORES, N_C, 9
    )
    out = np.empty((NCORES, N_C * 9), np.float32)
    pos = 0
    for m, off in zip(MS, np.concatenate([[0], np.cumsum(MS)[:-1]])):
        off = int(off)
        chunk = flat[:, off * P : (off + m) * P, :].reshape(NCORES, P, m, 9)
        sz = P * m * 9
        out[:, pos : pos + sz] = (
            chunk.transpose(0, 1, 3, 2)[:, :, PERM, :].reshape(NCORES, sz)
        )
        pos += sz
    return out


def _host_unpack(res_list) -> np.ndarray:
    out = np.empty((B, 3), np.float32)
    o = out.reshape(NCORES, N_C, 3)
    for i, r in enumerate(res_list):
        y = r["twist_out"]
        pos = 0
        for m, off in zip(MS, np.concatenate([[0], np.cumsum(MS)[:-1]])):
            off = int(off)
            sz = P * m * 3
            blk = y[pos : pos + sz].reshape(P, 3, m)
            o[i, off * P : (off + m) * P, :] = blk.transpose(0, 2, 1).reshape(
                P * m, 3
            )
            pos += sz
    return out


def kernel(mat_batch: np.ndarray) -> np.ndarray:
    if not _NC_CACHE:
        _NC_CACHE.append(_build_kernel())
    nc = _NC_CACHE[0]

    packed = _host_pack(mat_batch)
    in_maps = [{"mat_in": packed[i]} for i in range(NCORES)]
    res = run_bass_kernel_spmd(nc, in_maps, core_ids=list(range(NCORES)))
    return _host_unpack(res.results)


# revision 14
# speedup vs baseline: 1.0252x; 1.0252x over previous
"""Mat2Twist Trainium2 kernel: batch of 3x3 rotation matrices -> twist vectors.

For each matrix R:  tr = trace(R); x = (tr-1)/2 = cos(theta)
  theta = arccos(x);  w = [R21-R12, R02-R20, R10-R01]  (|w| = 2 sin theta)
  out = theta/(2 sin theta) * w

arccos via the Abramowitz-Stegun-style split
  arccos(x) = pi/2 + sign(x)*(arccos(|x|) - pi/2),
  arccos(|x|) = sqrt(1-|x|) * q(|x|),  q deg-2 minimax (err ~1e-3)
so every activation (Abs, Sign, Square, Sqrt) lives in ONE ACT table set
("sqrt_and_others") -- a Square/Ln/Exp/Arctan mix would force ~2
ACT_TABLE_LOADs (1.3us each) per chunk and make the scalar engine the
pipeline straggler.  1/sin(theta) uses the bitwise-NOT trick: for s>0,
z = s*bitcast(~bits(s)) always lands in [-4.5,-4], and a single
Chebyshev-tuned factor k*(z-A0)*bitcast(~bits(s)) ~= 1/s to 1.7e-3
(output gate is 2e-2) -- two DVE ops, no iterative divide.  All
remaining constants fold into existing scale/scalar immediate slots.

Per chunk (m matrices per partition), tile X = [minu(3m)|subt(3m)|R00|R11|R22]:
  gp:  tr = R00+R11+R22                       [GpSimd TT x2]
  act: ax=Abs(.5tr-.5) sg=Sign(.5tr-.5) v=Square(.5tr-.5)   (x=(tr-1)/2)
       qq=Square(SQ*ax+B2)  sn=Sqrt(1-v)  S=Sqrt(1-ax)  [6 ACT, one set]
  dve: w  = minu-subt (in place)
       nx = bitcast(bits(sn) ^ -1)            [int ts]
       z  = sn*nx                             [TT]
       u  = (qq+KQ)*S       = arccos(|x|)
       p  = (u-pi/2)*sg     = theta - pi/2
       A  = (p+pi/2)*nx     = theta*nx
       P  = (z-A0)*A        = theta/(k*sin theta)
       out_j = (HALF_K*P)*w_j  as one STT over [P,3,m], P broadcast
  out-DMA on the ACT ring, inputs on the SP ring.

The six small intermediates live as slices of ONE scratch tile per
chunk (2 pool allocations per chunk, not 7): every allocation costs a
per-engine event-semaphore round that the idle engines replay in a
multi-microsecond lockstep walk at kernel teardown.

Software-pipelined emission, skew: dma(i)@i -> trace@i+1 -> acts@i+2 ->
dve@i+3 -> out-dma trigger@i+4 (trigger emitted at the top of its
iteration so it never queues behind that iteration's ACT block).  Tail
chunks shrink (256,128,128) to cut pipeline-drain latency after the
last input byte lands.
"""

import numpy as np

import concourse.bass as bass
import concourse.mybir as mybir
from concourse.tile import TileContext
from concourse.bass_utils import run_bass_kernel_spmd

B = 4194304
NCORES = 8
P = 128
N_C = B // NCORES        # 524288 matrices per core
MPP = N_C // P           # 4096 matrices per partition
MS = [512] * 7 + [256] + [128] * 2   # per-chunk matrices per partition
assert sum(MS) == MPP

# component order in DRAM (flat 3x3 index): minuends, subtrahends, diagonal
PERM = [7, 2, 3, 5, 6, 1, 0, 4, 8]

F32 = mybir.dt.float32
I32 = mybir.dt.int32
ACT = mybir.ActivationFunctionType
ALU = mybir.AluOpType
PI_2 = float(np.pi / 2.0)
MAXM = max(MS)

# deg-2 minimax fit of q(t) = arccos(t)/sqrt(1-t) on [0, cos(0.1)],
# written as q = (SQ*t + B2)^2 + KQ so one ACT Square evaluates it.
SQ = 0.21443806949144176
B2 = -0.46791288034992956
KQ = 1.3508104959051634
# ~x reciprocal: rs ~= K_R*(z - A0)*bitcast(~bits(sn)), z = sn*bitcast(~bits(sn))
A0 = -8.5
K_R = -0.05545927
HALF_K = 0.5 * K_R


def _split_multi_waits(nc):
    """This container's walrus build rejects >1 sem-wait per instruction
    ("Too many sync wait commands"); hoist extras onto preceding NOPs."""
    for f in nc.m.functions:
        for blk in f.blocks:
            il = blk.instructions
            new = []
            for ins in il:
                si = ins.sync_info
                if si is not None and si.on_wait is not None and len(si.on_wait) > 1:
                    waits = list(si.on_wait)
                    for j, w in enumerate(waits[:-1]):
                        nop = mybir.InstNoOp(name=f"{ins.name}-ws{j}", engine=ins.engine)
                        nop.sync_info = mybir.SyncInfo(on_wait=[w], on_update=[])
                        new.append(nop)
                    ins.sync_info = mybir.SyncInfo(
                        on_wait=[waits[-1]], on_update=list(si.on_update or [])
                    )
                new.append(ins)
            il[:] = new


def _build_kernel():
    nc = bass.Bass()
    # extra const APs for activation biases (memsets + one barrier before
    # TileContext, same as the built-ins).
    for val in (B2, -0.5):
        t = nc.alloc_sbuf_tensor(f"const-f32-{val}", [128, 1], F32)
        nc.gpsimd.memset(t.ap(), val)
        nc.const_aps.aps[(F32, val)] = t.ap()
    nc.all_engine_barrier()

    x_in = nc.dram_tensor("mat_in", [N_C * 9], F32, kind="ExternalInput")
    y_out = nc.dram_tensor("twist_out", [N_C * 3], F32, kind="ExternalOutput")

    n = len(MS)
    offs = [0] + list(np.cumsum(MS)[:-1].astype(int))

    with TileContext(nc) as tc:
        with tc.tile_pool(name="xp", bufs=7) as xp, \
             tc.tile_pool(name="sp", bufs=4) as sp:

            X_, tr_, ax_, sg_, v_, qq_, nx_ = {}, {}, {}, {}, {}, {}, {}

            def dma_in(i):
                m = MS[i]
                base = offs[i] * P * 9
                X_[i] = xp.tile([P, 9 * MAXM], F32, tag="X", name=f"X{i}")[:, : 9 * m]
                nc.sync.dma_start(
                    out=X_[i],
                    in_=x_in[base : base + P * 9 * m].rearrange("(p n) -> p n", p=P),
                )

            def gp_trace(i):
                m = MS[i]
                X = X_[i]
                # one scratch tile per chunk; slices: tr|ax|sg|v|qq|nx
                s = sp.tile([P, 6 * MAXM], F32, tag="s", name=f"s{i}")
                tr_[i] = s[:, 0 * MAXM : 0 * MAXM + m]
                ax_[i] = s[:, 1 * MAXM : 1 * MAXM + m]
                sg_[i] = s[:, 2 * MAXM : 2 * MAXM + m]
                v_[i] = s[:, 3 * MAXM : 3 * MAXM + m]
                qq_[i] = s[:, 4 * MAXM : 4 * MAXM + m]
                nx_[i] = s[:, 5 * MAXM : 5 * MAXM + m]
                tr = tr_[i]
                nc.gpsimd.tensor_add(
                    out=tr, in0=X[:, 6 * m : 7 * m], in1=X[:, 7 * m : 8 * m]
                )
                nc.gpsimd.tensor_add(out=tr, in0=tr, in1=X[:, 8 * m : 9 * m])

            def act_block(i):
                tr, ax, sg, v, qq = tr_[i], ax_[i], sg_[i], v_[i], qq_[i]
                nc.scalar.activation(ax, tr, ACT.Abs, scale=0.5, bias=-0.5)
                nc.scalar.activation(sg, tr, ACT.Sign, scale=0.5, bias=-0.5)
                nc.scalar.activation(v, tr, ACT.Square, scale=0.5, bias=-0.5)
                nc.scalar.activation(qq, ax, ACT.Square, scale=SQ, bias=B2)
                # in place: sn over v, S over ax (qq already consumed ax)
                nc.scalar.activation(v, v, ACT.Sqrt, scale=-1.0, bias=1.0)
                nc.scalar.activation(ax, ax, ACT.Sqrt, scale=-1.0, bias=1.0)

            def dve_block(i):
                m = MS[i]
                X = X_[i]
                # w = minu - subt, in place in X
                nc.vector.tensor_sub(
                    out=X[:, 0 : 3 * m], in0=X[:, 0 : 3 * m], in1=X[:, 3 * m : 6 * m]
                )
                sn, nx, qq = v_[i], nx_[i], qq_[i]
                # nx = bitcast(bits(sn) ^ -1) (~= -C/sn); z = sn*nx in [-4.5,-4]
                nc.vector.tensor_scalar(
                    out=nx.bitcast(I32), in0=sn.bitcast(I32),
                    scalar1=-1, scalar2=None, op0=ALU.bitwise_xor,
                )
                nc.vector.tensor_mul(out=sn, in0=sn, in1=nx)  # z over sn
                # u = (qq + KQ) * S = arccos(|x|)
                nc.vector.scalar_tensor_tensor(
                    out=qq, in0=qq, scalar=KQ, in1=ax_[i],
                    op0=ALU.add, op1=ALU.mult,
                )
                # p = (u - pi/2) * sign(x) = theta - pi/2
                nc.vector.scalar_tensor_tensor(
                    out=qq, in0=qq, scalar=PI_2, in1=sg_[i],
                    op0=ALU.subtract, op1=ALU.mult,
                )
                # A = (p + pi/2) * nx = theta * nx
                nc.vector.scalar_tensor_tensor(
                    out=qq, in0=qq, scalar=PI_2, in1=nx,
                    op0=ALU.add, op1=ALU.mult,
                )
                # P = (z - A0) * A = theta/(K_R*sin theta)
                nc.vector.scalar_tensor_tensor(
                    out=qq, in0=sn, scalar=A0, in1=qq,
                    op0=ALU.subtract, op1=ALU.mult,
                )
                # out_j = (HALF_K * P) * w_j, P broadcast over the 3 w-blocks
                bcast = qq.rearrange("p (o n) -> p o n", o=1).to_broadcast((P, 3, m))
                w3 = X[:, 0 : 3 * m].rearrange("p (k n) -> p k n", k=3)
                nc.vector.scalar_tensor_tensor(
                    out=w3, in0=bcast, scalar=HALF_K, in1=w3,
                    op0=ALU.mult, op1=ALU.mult,
                )

            def out_dma(i):
                m = MS[i]
                dst = y_out[offs[i] * P * 3 : (offs[i] + m) * P * 3].rearrange(
                    "(p n) -> p n", p=P
                )
                nc.scalar.dma_start(out=dst, in_=X_[i][:, 0 : 3 * m])

            def valid(j):
                return 0 <= j < n

            # software-pipelined emission; skew in iterations:
            # dma(i)@i, trace@i+1, acts@i+2, dve@i+3, out-dma@i+4 (trigger
            # emitted at the top of the iteration so the ACT-ring trigger
            # never queues behind that iteration's ACT block)
            for i in range(n + 4):
                if valid(i):
                    dma_in(i)
                if valid(i - 4):
                    out_dma(i - 4)
                if valid(i - 1):
                    gp_trace(i - 1)
                if valid(i - 3):
                    dve_block(i - 3)
                if valid(i - 2):
                    act_block(i - 2)

    _split_multi_waits(nc)
    return nc


_NC_CACHE = []


def _host_pack(mat_batch: np.ndarray) -> np.ndarray:
    """[B,3,3] -> [NCORES, N_C*9] tile-major/component-major PERM layout."""
    flat = np.ascontiguousarray(mat_batch, dtype=np.float32).reshape(
        NCORES, N_C, 9
    )
    out = np.empty((NCORES, N_C * 9), np.float32)
    pos = 0
    for m, off in zip(MS, np.concatenate([[0], np.cumsum(MS)[:-1]])):
        off = int(off)
        chunk = flat[:, off * P : (off + m) * P, :].reshape(NCORES, P, m, 9)
        sz = P * m * 9
        out[:, pos : pos + sz] = (
            chunk.transpose(0, 1, 3, 2)[:, :, PERM, :].reshape(NCORES, sz)
        )
        pos += sz
    return out


def _host_unpack(res_list) -> np.ndarray:
    out = np.empty((B, 3), np.float32)
    o = out.reshape(NCORES, N_C, 3)
    for i, r in enumerate(res_list):
        y = r["twist_out"]
        pos = 0
        for m, off in zip(MS, np.concatenate([[0], np.cumsum(MS)[:-1]])):
            off = int(off)
            sz = P * m * 3
            blk = y[pos : pos + sz].reshape(P, 3, m)
            o[i, off * P : (off + m) * P, :] = blk.transpose(0, 2, 1).reshape(
                P * m, 3
            )
            pos += sz
    return out


def kernel(mat_batch: np.ndarray) -> np.ndarray:
    if not _NC_CACHE:
        _NC_CACHE.append(_build_kernel())
    nc = _NC_CACHE[0]

    packed = _host_pack(mat_batch)
    in_maps = [{"mat_in": packed[i]} for i in range(NCORES)]
    res = run_bass_kernel_spmd(nc, in_maps, core_ids=list(range(NCORES)))
    return _host_unpack(res.results)
